# revision 1
# baseline (speedup 1.0000x reference)
"""BoxFilter (9x9 box sum with edge clamping) on 8 Trainium2 NeuronCores.

Reference semantics (B, C, H, W fp32, r=4):
    out = diff_y(cumsum_W(diff_x(cumsum_H(x))))
i.e. a separable 9-wide box *sum* along H then W, with windows truncated at
the image borders.

Strategy:
  - Shard data-parallel over batch: B=8 -> one (3, 1080, 1920) image per core.
  - Per core, 27 tiles (3 channels x 9 blocks of 120 output rows). Each tile
    loads 128 input rows (+-4 halo) x full W.
  - W-direction box: one DVE tensor_tensor_scan implementing
        S[w] = S[w-1] + x[w+4] - x[w-5]
    over a zero-padded row. The row is padded with 2r+1 zeros on the left and
    r on the right, and the scan starts r steps early from initial=0 so the
    window warm-up happens inside the scan (no separate init reduce).
  - H-direction box: TensorE matmul with a constant 0/1 banded matrix
    [K=128, M=120] (one variant each for top / interior / bottom blocks).
  - PSUM -> SBUF via ScalarE copy; loads on the SP HWDGE ring, stores on the
    ACT HWDGE ring.
"""

import sys

if "/opt/trn_rl_repo" not in sys.path:
    sys.path.insert(0, "/opt/trn_rl_repo")

import numpy as np

B, C, H, W = 8, 3, 1080, 1920
R = 4
BLK = 120          # output rows per tile
NBLK = H // BLK    # 9
LPAD = 2 * R + 1   # 9 left zeros
XW = LPAD + W + R  # padded row width (1933)
SCN = W + R        # scan length (1924); outputs [R:] are S[0..W-1]
N_CHUNKS = (W + 511) // 512  # matmul N<=512 fp32 (PSUM bank)


def _band_matrices() -> np.ndarray:
    """[128, 3*BLK] fp32: the three 0/1 banded H-box matrices, side by side.

    out[m, n] = sum_k band[k, m] * in[k, n]; column m holds the taps for
    output row m of the block.
    """
    b0 = np.zeros((128, BLK), np.float32)   # first block: rows 0..127 loaded
    b1 = np.zeros((128, BLK), np.float32)   # interior: rows h0-4..h0+123
    b2 = np.zeros((128, BLK), np.float32)   # last block: rows H-128..H-1
    for m in range(BLK):
        b0[max(0, m - R): m + R + 1, m] = 1.0
        b1[m: m + 2 * R + 1, m] = 1.0
        b2[m + R: min(m + 3 * R, 127) + 1, m] = 1.0
    return np.concatenate([b0, b1, b2], axis=1)


def _build_nc():
    import concourse.tile as tile
    from concourse import bacc, mybir

    f32 = mybir.dt.float32
    nc = bacc.Bacc("TRN2", target_bir_lowering=False, debug=False)
    x_d = nc.dram_tensor("x", [C, H, W], f32, kind="ExternalInput").ap()
    out_d = nc.dram_tensor("out", [C, H, W], f32, kind="ExternalOutput").ap()
    bands_d = nc.inline_tensor(_band_matrices(), name="bands").ap()

    with tile.TileContext(nc) as tc:
        _tile_body(tc, out_d, x_d, bands_d, f32, mybir)
    nc.compile()
    return nc


# float32r matmuls would run 1 HW pass instead of fp32's LOW_HIGH 2 and save
# ~4us, but round the scan output to ~12 mantissa bits (absmax err 6.6e-3,
# ~50x the reference's own fp32 envelope of 1.3e-4). Not worth the risk
# against an envelope-based correctness gate.
USE_F32R = False


def _tile_body(tc, out_d, x_d, bands_d, f32, mybir):
    nc = tc.nc
    add = mybir.AluOpType.add
    sub = mybir.AluOpType.subtract
    f32r = mybir.dt.float32r
    mm_dt = f32r if USE_F32R else f32

    with (
        tc.tile_pool(name="bands", bufs=1) as bands_pool,
        tc.tile_pool(name="xp", bufs=8) as xpool,
        tc.tile_pool(name="wb", bufs=7) as wpool,
        tc.tile_pool(name="ot", bufs=7) as opool,
        tc.tile_pool(name="ps", bufs=8, space="PSUM") as pspool,
    ):
        bands = bands_pool.tile([128, 3 * BLK], mm_dt)
        first = True

        for c in range(C):
            for t in range(NBLK):
                h0 = t * BLK
                if t == 0:
                    r0, bi = 0, 0
                elif t == NBLK - 1:
                    r0, bi = H - 128, 2
                else:
                    r0, bi = h0 - R, 1

                xp = xpool.tile([128, XW], f32)
                nc.gpsimd.memset(xp[:, 0:LPAD], 0.0)
                nc.gpsimd.memset(xp[:, LPAD + W: XW], 0.0)
                nc.sync.dma_start(
                    out=xp[:, LPAD: LPAD + W], in_=x_d[c, r0: r0 + 128, :]
                )
                if first:
                    # bands aren't needed until the first matmul; issue their
                    # DMA on the ACT ring (idle at fill time) so the SP ring
                    # streams tile loads back-to-back. (0/1 values are exact
                    # in f32r, so the bitcast is value-preserving.)
                    nc.scalar.dma_start(
                        out=bands[:, :], in_=bands_d[:, :].bitcast(bands.dtype)
                    )
                    first = False

                # scan t=0..SCN-1: state = (xp[t+LPAD-R] ... ) computing
                # S[w] = S[w-1] + x[w+4] - x[w-5] from w=-R with state 0;
                # wb[:, R:] holds S[0..W-1]
                wb = wpool.tile([128, SCN], mm_dt)
                nc.vector.tensor_tensor_scan(
                    out=wb[:, :],
                    data0=xp[:, LPAD: LPAD + SCN],
                    data1=xp[:, 0:SCN],
                    initial=0.0,
                    op0=add,
                    op1=sub,
                )

                # one single-bank PSUM tile per 512-col chunk: PE rotates
                # through 8 banks and never waits on a whole-tile evacuation
                band = bands[:, bi * BLK: (bi + 1) * BLK]
                ot = opool.tile([BLK, W], f32)
                # For the very last tile, DVE is idle (its scan train is
                # done): evacuate PSUM on DVE and ship the store in halves
                # to shorten the end-of-kernel drain.
                last = c == C - 1 and t == NBLK - 1
                for j in range(N_CHUNKS):
                    n0 = j * 512
                    nw = min(512, W - n0)
                    psj = pspool.tile([BLK, 512], f32)
                    nc.tensor.matmul(
                        out=psj[:, 0:nw],
                        lhsT=band,
                        rhs=wb[:, R + n0: R + n0 + nw],
                        start=True,
                        stop=True,
                    )
                    if last:
                        nc.vector.tensor_copy(
                            ot[:, n0: n0 + nw], psj[:, 0:nw]
                        )
                        if j == 1:
                            nc.scalar.dma_start(
                                out=out_d[c, h0: h0 + BLK, 0:1024],
                                in_=ot[:, 0:1024],
                            )
                        elif j == N_CHUNKS - 1:
                            nc.scalar.dma_start(
                                out=out_d[c, h0: h0 + BLK, 1024:W],
                                in_=ot[:, 1024:W],
                            )
                    else:
                        nc.scalar.copy(out=ot[:, n0: n0 + nw], in_=psj[:, 0:nw])
                if not last:
                    # stores on the ACT HWDGE ring so they don't head-of-line
                    # block loads on the SP ring
                    nc.scalar.dma_start(
                        out=out_d[c, h0: h0 + BLK, :], in_=ot[:, :]
                    )


_NC = None


def _get_nc():
    global _NC
    if _NC is None:
        _NC = _build_nc()
    return _NC


def run(x: np.ndarray, trace: bool = False, trace_cores=None):
    """Run the kernel on all 8 cores. Returns (out, BassKernelResults)."""
    from concourse.bass_utils import run_bass_kernel_spmd

    nc = _get_nc()
    x = np.ascontiguousarray(np.asarray(x, dtype=np.float32))
    assert x.shape == (B, C, H, W), x.shape
    in_maps = [{"x": x[b]} for b in range(B)]
    if trace and trace_cores is None:
        trace_cores = [0, 7]
    res = run_bass_kernel_spmd(
        nc, in_maps, core_ids=list(range(B)), trace=trace, trace_cores=trace_cores
    )
    out = np.stack([res.results[b]["out"] for b in range(B)], axis=0)
    return out, res


def kernel(x: np.ndarray, r) -> np.ndarray:
    assert int(np.asarray(r)) == R, f"kernel hardcodes r={R}, got {r}"
    out, _ = run(x, trace=False)
    return out



# revision 2
# speedup vs baseline: 1.3975x; 1.3975x over previous
"""BoxFilter (9x9 box sum with edge clamping) on 8 Trainium2 NeuronCores.

Reference semantics (B, C, H, W fp32, r=4):
    out = diff_y(cumsum_W(diff_x(cumsum_H(x))))
i.e. a separable 9-wide box *sum* along H then W, with windows truncated at
the image borders.

Strategy (v2 — fp16 I/O, PE-assisted W-box):
  - Shard data-parallel over batch: B=8 -> one (3, 1080, 1920) image per core.
  - HBM I/O in fp16 (host converts): halves DMA traffic vs fp32. The
    correctness budget (rel 2e-2 of output scale ~48) dwarfs fp16 rounding.
  - Factor the 9-tap W-box as box3 (*) comb3:  ones(9) = ones(3) conv
    {delta(-3), delta(0), delta(+3)}.
      * DVE computes box3 with two shifted tensor_tensor adds (fp16 2x_1p
        mode: 0.5 cyc/elem). The old tensor_tensor_scan ran 2 cyc/elem with
        no fast mode and dominated the kernel at ~110us.
      * PE applies the H-box band matrix three times per PSUM chunk with
        rhs shifted by {0, 3, 6} columns, accumulating in PSUM. fp16
        matmul is 1 cyc/row (fp32 was 4), so 3 passes still beat fp32.
  - Zero-padded rows (4 left, 4 right) make the border truncation fall out
    of the math for both box3 and the comb shifts.
  - PSUM -> SBUF (fp32->fp16) on ScalarE; loads on the SP HWDGE ring,
    stores on GpSimd SWDGE so neither HWDGE ring carries both directions.
  - Per core, 27 tiles (3 channels x 9 blocks of 120 output rows); each
    tile loads 128 input rows (+-4 halo) x full W. The H-box is a constant
    0/1 banded matrix (top / interior / bottom variants).
"""

import sys

if "/opt/trn_rl_repo" not in sys.path:
    sys.path.insert(0, "/opt/trn_rl_repo")

import numpy as np

B, C, H, W = 8, 3, 1080, 1920
R = 4
BLK = 120          # output rows per tile
NBLK = H // BLK    # 9
LP = 4             # left zero pad (x needed on [-4, W+3])
XW = W + 2 * LP    # padded row width (1928)
W3 = W + 6         # box3 row width: box3[w] for w in [-3, W+2]
N_CHUNKS = (W + 511) // 512  # PSUM chunks (512 fp32 = one bank)


def _band_matrices() -> np.ndarray:
    """[128, 3*BLK] fp16: the three 0/1 banded H-box matrices, side by side.

    out[m, n] = sum_k band[k, m] * in[k, n]; column m holds the taps for
    output row m of the block.
    """
    b0 = np.zeros((128, BLK), np.float16)   # first block: rows 0..127 loaded
    b1 = np.zeros((128, BLK), np.float16)   # interior: rows h0-4..h0+123
    b2 = np.zeros((128, BLK), np.float16)   # last block: rows H-128..H-1
    for m in range(BLK):
        b0[max(0, m - R): m + R + 1, m] = 1.0
        b1[m: m + 2 * R + 1, m] = 1.0
        b2[m + R: min(m + 3 * R, 127) + 1, m] = 1.0
    return np.concatenate([b0, b1, b2], axis=1)


def _build_nc():
    import concourse.tile as tile
    from concourse import bacc, mybir

    f16 = mybir.dt.float16
    nc = bacc.Bacc("TRN2", target_bir_lowering=False, debug=False)
    x_d = nc.dram_tensor("x", [C, H, W], f16, kind="ExternalInput").ap()
    out_d = nc.dram_tensor("out", [C, H, W], f16, kind="ExternalOutput").ap()
    bands_d = nc.inline_tensor(_band_matrices(), name="bands").ap()

    with tile.TileContext(nc) as tc:
        _tile_body(tc, out_d, x_d, bands_d, mybir)
    nc.compile()
    return nc


def _tile_body(tc, out_d, x_d, bands_d, mybir):
    nc = tc.nc
    add = mybir.AluOpType.add
    f16 = mybir.dt.float16
    f32 = mybir.dt.float32

    with (
        tc.tile_pool(name="bands", bufs=1) as bands_pool,
        tc.tile_pool(name="xp", bufs=8) as xpool,
        tc.tile_pool(name="t1", bufs=3) as tpool,
        tc.tile_pool(name="wb", bufs=4) as wpool,
        tc.tile_pool(name="ot", bufs=4) as opool,
        tc.tile_pool(name="ps", bufs=8, space="PSUM") as pspool,
    ):
        bands = bands_pool.tile([128, 3 * BLK], f16)
        first = True
        tile_idx = 0

        for c in range(C):
            for t in range(NBLK):
                h0 = t * BLK
                if t == 0:
                    r0, bi = 0, 0
                elif t == NBLK - 1:
                    r0, bi = H - 128, 2
                else:
                    r0, bi = h0 - R, 1

                xp = xpool.tile([128, XW], f16)
                if tile_idx < 8:
                    # pool buffers rotate round-robin; pads stay zero after
                    # the first pass since DMA only writes the middle
                    nc.gpsimd.memset(xp[:, 0:LP], 0.0)
                    nc.gpsimd.memset(xp[:, LP + W: XW], 0.0)
                nc.sync.dma_start(
                    out=xp[:, LP: LP + W], in_=x_d[c, r0: r0 + 128, :]
                )
                if first:
                    # bands aren't needed until the first matmul; use the
                    # ACT ring (idle at fill time)
                    nc.scalar.dma_start(out=bands[:, :], in_=bands_d[:, :])
                    first = False

                # box3 along W: wb[:, j] = x[j-4] + x[j-3] + x[j-2]
                # (j indexes box3[w] at w = j-3; x zero-padded by LP=4)
                t1 = tpool.tile([128, W3], f16)
                nc.vector.tensor_tensor(
                    out=t1[:, :], in0=xp[:, 0:W3], in1=xp[:, 1:1 + W3], op=add
                )
                wb = wpool.tile([128, W3], f16)
                nc.vector.tensor_tensor(
                    out=wb[:, :], in0=t1[:, :], in1=xp[:, 2:2 + W3], op=add
                )

                # H-box x comb3: out[:, w] = sum_{s in {0,3,6}} band.T @
                # wb[:, w+s]  (wb[w+0..6] = box3 at w-3, w, w+3)
                band = bands[:, bi * BLK: (bi + 1) * BLK]
                ot = opool.tile([BLK, W], f16)
                for j in range(N_CHUNKS):
                    n0 = j * 512
                    nw = min(512, W - n0)
                    psj = pspool.tile([BLK, 512], f32)
                    for si, s in enumerate((0, 3, 6)):
                        nc.tensor.matmul(
                            out=psj[:, 0:nw],
                            lhsT=band,
                            rhs=wb[:, n0 + s: n0 + s + nw],
                            start=(si == 0),
                            stop=(si == 2),
                        )
                    nc.scalar.copy(out=ot[:, n0: n0 + nw], in_=psj[:, 0:nw])
                # stores on GpSimd SWDGE: keeps the ACT HWDGE ring free for
                # PSUM evacuation and the SP ring free for loads
                nc.gpsimd.dma_start(out=out_d[c, h0: h0 + BLK, :], in_=ot[:, :])
                tile_idx += 1


_NC = None


def _get_nc():
    global _NC
    if _NC is None:
        _NC = _build_nc()
    return _NC


def run(x: np.ndarray, trace: bool = False, trace_cores=None):
    """Run the kernel on all 8 cores. Returns (out, BassKernelResults)."""
    from concourse.bass_utils import run_bass_kernel_spmd

    nc = _get_nc()
    x = np.asarray(x)
    assert x.shape == (B, C, H, W), x.shape
    x16 = np.ascontiguousarray(x.astype(np.float16))
    in_maps = [{"x": x16[b]} for b in range(B)]
    if trace and trace_cores is None:
        trace_cores = [0, 7]
    res = run_bass_kernel_spmd(
        nc, in_maps, core_ids=list(range(B)), trace=trace, trace_cores=trace_cores
    )
    out = np.stack([res.results[b]["out"] for b in range(B)], axis=0)
    return out.astype(np.float32), res


def kernel(x: np.ndarray, r) -> np.ndarray:
    assert int(np.asarray(r)) == R, f"kernel hardcodes r={R}, got {r}"
    out, _ = run(x, trace=False)
    return out


# revision 7
# speedup vs baseline: 1.4019x; 1.0032x over previous
"""BoxFilter (9x9 box sum with edge clamping) on 8 Trainium2 NeuronCores.

Reference semantics (B, C, H, W fp32, r=4):
    out = diff_y(cumsum_W(diff_x(cumsum_H(x))))
i.e. a separable 9-wide box *sum* along H then W, with windows truncated at
the image borders.

Strategy (v2 — fp16 I/O, PE-assisted W-box):
  - Shard data-parallel over batch: B=8 -> one (3, 1080, 1920) image per core.
  - HBM I/O in fp16 (host converts): halves DMA traffic vs fp32. The
    correctness budget (rel 2e-2 of output scale ~48) dwarfs fp16 rounding.
  - Factor the 9-tap W-box as box3 (*) comb3:  ones(9) = ones(3) conv
    {delta(-3), delta(0), delta(+3)}.
      * DVE computes box3 with two shifted tensor_tensor adds (fp16 2x_1p
        mode: 0.5 cyc/elem). The old tensor_tensor_scan ran 2 cyc/elem with
        no fast mode and dominated the kernel at ~110us.
      * PE applies the H-box band matrix three times per PSUM chunk with
        rhs shifted by {0, 3, 6} columns, accumulating in PSUM. fp16
        matmul is 1 cyc/row (fp32 was 4), so 3 passes still beat fp32.
  - Zero-padded rows (4 left, 4 right) make the border truncation fall out
    of the math for both box3 and the comb shifts.
  - PSUM -> SBUF (fp32->fp16) on ScalarE; loads on the SP HWDGE ring,
    stores on GpSimd SWDGE so neither HWDGE ring carries both directions.
  - Per core, 27 tiles (3 channels x 9 blocks of 120 output rows); each
    tile loads 128 input rows (+-4 halo) x full W. The H-box is a constant
    0/1 banded matrix (top / interior / bottom variants).
"""

import sys

if "/opt/trn_rl_repo" not in sys.path:
    sys.path.insert(0, "/opt/trn_rl_repo")

import numpy as np

B, C, H, W = 8, 3, 1080, 1920
R = 4
BLK = 120          # output rows per tile
NBLK = H // BLK    # 9
LP = 4             # left zero pad (x needed on [-4, W+3])
XW = W + 2 * LP    # padded row width (1928)
W3 = W + 6         # box3 row width: box3[w] for w in [-3, W+2]
N_CHUNKS = (W + 511) // 512  # PSUM chunks (512 fp32 = one bank)


def _band_matrices() -> np.ndarray:
    """[128, 3*BLK] fp16: the three 0/1 banded H-box matrices, side by side.

    out[m, n] = sum_k band[k, m] * in[k, n]; column m holds the taps for
    output row m of the block.
    """
    b0 = np.zeros((128, BLK), np.float16)   # first block: rows 0..127 loaded
    b1 = np.zeros((128, BLK), np.float16)   # interior: rows h0-4..h0+123
    b2 = np.zeros((128, BLK), np.float16)   # last block: rows H-128..H-1
    for m in range(BLK):
        b0[max(0, m - R): m + R + 1, m] = 1.0
        b1[m: m + 2 * R + 1, m] = 1.0
        b2[m + R: min(m + 3 * R, 127) + 1, m] = 1.0
    return np.concatenate([b0, b1, b2], axis=1)


def _build_nc():
    import concourse.tile as tile
    from concourse import bacc, mybir

    f16 = mybir.dt.float16
    nc = bacc.Bacc("TRN2", target_bir_lowering=False, debug=False)
    x_d = nc.dram_tensor("x", [C, H, W], f16, kind="ExternalInput").ap()
    out_d = nc.dram_tensor("out", [C, H, W], f16, kind="ExternalOutput").ap()
    bands_d = nc.inline_tensor(_band_matrices(), name="bands").ap()

    with tile.TileContext(nc) as tc:
        _tile_body(tc, out_d, x_d, bands_d, mybir)
    nc.compile()
    return nc


def _tile_body(tc, out_d, x_d, bands_d, mybir):
    nc = tc.nc
    add = mybir.AluOpType.add
    f16 = mybir.dt.float16
    f32 = mybir.dt.float32

    with (
        tc.tile_pool(name="bands", bufs=1) as bands_pool,
        tc.tile_pool(name="xp", bufs=8) as xpool,
        tc.tile_pool(name="t1", bufs=3) as tpool,
        tc.tile_pool(name="wb", bufs=4) as wpool,
        tc.tile_pool(name="ot", bufs=4) as opool,
        tc.tile_pool(name="ps", bufs=4, space="PSUM") as pspool,
    ):
        bands = bands_pool.tile([128, 3 * BLK], f16)
        first = True
        tile_idx = 0

        for c in range(C):
            for t in range(NBLK):
                h0 = t * BLK
                if t == 0:
                    r0, bi = 0, 0
                elif t == NBLK - 1:
                    r0, bi = H - 128, 2
                else:
                    r0, bi = h0 - R, 1

                xp = xpool.tile([128, XW], f16)
                if tile_idx < 8:
                    # pool buffers rotate round-robin; pads stay zero after
                    # the first pass since DMA only writes the middle
                    nc.gpsimd.memset(xp[:, 0:LP], 0.0)
                    nc.gpsimd.memset(xp[:, LP + W: XW], 0.0)
                # alternate loads between the SP and ACT HWDGE rings so one
                # ring doesn't carry the whole 13.3 MB read stream
                load_eng = nc.sync if tile_idx % 2 == 0 else nc.scalar
                load_eng.dma_start(
                    out=xp[:, LP: LP + W], in_=x_d[c, r0: r0 + 128, :]
                )
                if first:
                    nc.sync.dma_start(out=bands[:, :], in_=bands_d[:, :])
                    first = False

                # box3 along W: wb[:, j] = x[j-4] + x[j-3] + x[j-2]
                # (j indexes box3[w] at w = j-3; x zero-padded by LP=4)
                t1 = tpool.tile([128, W3], f16)
                nc.vector.tensor_tensor(
                    out=t1[:, :], in0=xp[:, 0:W3], in1=xp[:, 1:1 + W3], op=add
                )
                wb = wpool.tile([128, W3], f16)
                nc.vector.tensor_tensor(
                    out=wb[:, :], in0=t1[:, :], in1=xp[:, 2:2 + W3], op=add
                )

                # H-box x comb3: out[:, w] = sum_{s in {0,3,6}} band.T @
                # wb[:, w+s]  (wb[w+0..6] = box3 at w-3, w, w+3)
                band = bands[:, bi * BLK: (bi + 1) * BLK]
                ot = opool.tile([BLK, W], f16)
                # two PSUM tiles of 2 banks each; matmuls still write
                # single-bank 512-col slices, but evacuation runs as one
                # wide ACTIVATE per 2-bank tile (fewer ACT dispatches)
                for half in range(2):
                    h0c = half * 1024
                    hw = min(1024, W - h0c)  # 1024 then 896
                    psj = pspool.tile([BLK, 1024], f32)
                    for sub in range(2):
                        n0 = h0c + sub * 512
                        if n0 >= W:
                            break
                        nw = min(512, W - n0)
                        for si, s in enumerate((0, 3, 6)):
                            nc.tensor.matmul(
                                out=psj[:, sub * 512: sub * 512 + nw],
                                lhsT=band,
                                rhs=wb[:, n0 + s: n0 + s + nw],
                                start=(si == 0),
                                stop=(si == 2),
                            )
                    nc.scalar.copy(
                        out=ot[:, h0c: h0c + hw], in_=psj[:, 0:hw]
                    )
                # stores on GpSimd SWDGE: keeps the ACT HWDGE ring free for
                # PSUM evacuation and the SP ring free for loads
                nc.gpsimd.dma_start(out=out_d[c, h0: h0 + BLK, :], in_=ot[:, :])
                tile_idx += 1


_NC = None


def _get_nc():
    global _NC
    if _NC is None:
        _NC = _build_nc()
    return _NC


def run(x: np.ndarray, trace: bool = False, trace_cores=None):
    """Run the kernel on all 8 cores. Returns (out, BassKernelResults)."""
    from concourse.bass_utils import run_bass_kernel_spmd

    nc = _get_nc()
    x = np.asarray(x)
    assert x.shape == (B, C, H, W), x.shape
    x16 = np.ascontiguousarray(x.astype(np.float16))
    in_maps = [{"x": x16[b]} for b in range(B)]
    if trace and trace_cores is None:
        trace_cores = [0, 7]
    res = run_bass_kernel_spmd(
        nc, in_maps, core_ids=list(range(B)), trace=trace, trace_cores=trace_cores
    )
    out = np.stack([res.results[b]["out"] for b in range(B)], axis=0)
    return out.astype(np.float32), res


def kernel(x: np.ndarray, r) -> np.ndarray:
    assert int(np.asarray(r)) == R, f"kernel hardcodes r={R}, got {r}"
    out, _ = run(x, trace=False)
    return out


# revision 8
# speedup vs baseline: 1.4109x; 1.0064x over previous
"""BoxFilter (9x9 box sum with edge clamping) on 8 Trainium2 NeuronCores.

Reference semantics (B, C, H, W fp32, r=4):
    out = diff_y(cumsum_W(diff_x(cumsum_H(x))))
i.e. a separable 9-wide box *sum* along H then W, with windows truncated at
the image borders.

Strategy (v4 — fp16 I/O, PE/DVE load-balanced W-box):
  - Shard data-parallel over batch: B=8 -> one (3, 1080, 1920) image per core.
  - HBM I/O in fp16 (host converts): halves DMA traffic vs fp32. The
    correctness budget (rel 2e-2 of output scale ~48) dwarfs fp16 rounding.
  - Per core, 27 tiles (3 channels x 9 blocks of 120 output rows); each
    tile loads 128 input rows (+-4 halo) x full W. The H-box is a constant
    0/1 banded matrix (top / interior / bottom variants) applied on PE.
  - W-box, two variants balanced across tiles:
      * comb path (most tiles): ones(9) = ones(3) conv {d(-3),d(0),d(+3)}.
        DVE computes box3 with two shifted tensor_tensor adds (fp16 2x_1p
        = 0.5 cyc/elem); PE applies the H-band 3x per 512-col group with
        rhs shifted by {0,3,6}, accumulating in PSUM (fp16 matmul is
        1 cyc/col, so 3 passes ~2.6us/tile).
      * scan path (a few tiles): the old DVE tensor_tensor_scan
        S[w] = S[w-1] + x[w+4] - x[w-5] (4.1us/tile, no fast mode) and a
        single PE pass. DVE cost ~2x comb, PE cost ~1/3: assigning ~3
        tiles to this path equalizes PE and DVE at ~64us each.
  - Zero-padded rows (9 left for scan warm-up, 4 right) give border
    truncation for free in both variants.
  - PSUM -> SBUF (fp32->fp16) as ONE wide ACTIVATE per tile on ScalarE
    ([120, 1920] over a 4-bank PSUM tile, bufs=2).
  - Loads: SP HWDGE ring mostly, every 3rd tile on the ACT ring; stores
    on GpSimd SWDGE; GpSimd is barred from PSUM so it only stores+memsets.
"""

import sys

if "/opt/trn_rl_repo" not in sys.path:
    sys.path.insert(0, "/opt/trn_rl_repo")

import numpy as np

B, C, H, W = 8, 3, 1080, 1920
R = 4
BLK = 120          # output rows per tile
NBLK = H // BLK    # 9
LP = 9             # left zero pad (scan warm-up needs 2r+1)
RP = 4             # right zero pad
XW = LP + W + RP   # padded row width (1933)
W3 = W + 6         # box3 row width: box3[w] for w in [-3, W+2]
SCN = W + R        # scan length; outputs [R:] are S[0..W-1]
N_TILES = C * NBLK
# tiles that take the scan path (DVE-heavy, PE-light) to balance engines
SCAN_TILES = frozenset({4, 13, 22})


def _band_matrices() -> np.ndarray:
    """[128, 3*BLK] fp16: the three 0/1 banded H-box matrices, side by side.

    out[m, n] = sum_k band[k, m] * in[k, n]; column m holds the taps for
    output row m of the block.
    """
    b0 = np.zeros((128, BLK), np.float16)   # first block: rows 0..127 loaded
    b1 = np.zeros((128, BLK), np.float16)   # interior: rows h0-4..h0+123
    b2 = np.zeros((128, BLK), np.float16)   # last block: rows H-128..H-1
    for m in range(BLK):
        b0[max(0, m - R): m + R + 1, m] = 1.0
        b1[m: m + 2 * R + 1, m] = 1.0
        b2[m + R: min(m + 3 * R, 127) + 1, m] = 1.0
    return np.concatenate([b0, b1, b2], axis=1)


def _build_nc():
    import concourse.tile as tile
    from concourse import bacc, mybir

    f16 = mybir.dt.float16
    nc = bacc.Bacc("TRN2", target_bir_lowering=False, debug=False)
    x_d = nc.dram_tensor("x", [C, H, W], f16, kind="ExternalInput").ap()
    out_d = nc.dram_tensor("out", [C, H, W], f16, kind="ExternalOutput").ap()
    bands_d = nc.inline_tensor(_band_matrices(), name="bands").ap()

    with tile.TileContext(nc) as tc:
        _tile_body(tc, out_d, x_d, bands_d, mybir)
    nc.compile()
    return nc


def _tile_body(tc, out_d, x_d, bands_d, mybir):
    nc = tc.nc
    add = mybir.AluOpType.add
    sub = mybir.AluOpType.subtract
    f16 = mybir.dt.float16
    f32 = mybir.dt.float32

    with (
        tc.tile_pool(name="bands", bufs=1) as bands_pool,
        tc.tile_pool(name="xp", bufs=8) as xpool,
        tc.tile_pool(name="t1", bufs=3) as tpool,
        tc.tile_pool(name="wb", bufs=4) as wpool,
        tc.tile_pool(name="ot", bufs=4) as opool,
        tc.tile_pool(name="ps", bufs=2, space="PSUM") as pspool,
    ):
        bands = bands_pool.tile([128, 3 * BLK], f16)
        first = True
        tile_idx = 0

        for c in range(C):
            for t in range(NBLK):
                h0 = t * BLK
                if t == 0:
                    r0, bi = 0, 0
                elif t == NBLK - 1:
                    r0, bi = H - 128, 2
                else:
                    r0, bi = h0 - R, 1

                xp = xpool.tile([128, XW], f16)
                if tile_idx < 8:
                    # pool buffers rotate round-robin; pads stay zero after
                    # the first pass since DMA only writes the middle
                    nc.gpsimd.memset(xp[:, 0:LP], 0.0)
                    nc.gpsimd.memset(xp[:, LP + W: XW], 0.0)
                # most loads on the SP ring; every 3rd on the ACT ring so
                # neither ring carries the whole 13.3 MB read stream
                load_eng = nc.scalar if tile_idx % 3 == 2 else nc.sync
                load_eng.dma_start(
                    out=xp[:, LP: LP + W], in_=x_d[c, r0: r0 + 128, :]
                )
                if first:
                    nc.sync.dma_start(out=bands[:, :], in_=bands_d[:, :])
                    first = False

                band = bands[:, bi * BLK: (bi + 1) * BLK]
                ot = opool.tile([BLK, W], f16)
                psj = pspool.tile([BLK, 2048], f32)

                if tile_idx in SCAN_TILES:
                    # scan path: W-box in one DVE scan, one PE pass
                    wb = wpool.tile([128, W3], f16)
                    nc.vector.tensor_tensor_scan(
                        out=wb[:, 0:SCN],
                        data0=xp[:, LP: LP + SCN],
                        data1=xp[:, 0:SCN],
                        initial=0.0,
                        op0=add,
                        op1=sub,
                    )
                    for q in range(4):
                        n0 = q * 512
                        nw = min(512, W - n0)
                        nc.tensor.matmul(
                            out=psj[:, n0: n0 + nw],
                            lhsT=band,
                            rhs=wb[:, R + n0: R + n0 + nw],
                            start=True,
                            stop=True,
                        )
                else:
                    # comb path: box3 on DVE (2 adds), 3 shifted PE passes
                    # wb[:, j] = box3 at w=j-3  =  x[j-4] + x[j-3] + x[j-2]
                    #          = xp[:, j+5] + xp[:, j+6] + xp[:, j+7]
                    t1 = tpool.tile([128, W3], f16)
                    nc.vector.tensor_tensor(
                        out=t1[:, :], in0=xp[:, 5:5 + W3],
                        in1=xp[:, 6:6 + W3], op=add,
                    )
                    wb = wpool.tile([128, W3], f16)
                    nc.vector.tensor_tensor(
                        out=wb[:, :], in0=t1[:, :],
                        in1=xp[:, 7:7 + W3], op=add,
                    )
                    for q in range(4):
                        n0 = q * 512
                        nw = min(512, W - n0)
                        for si, s in enumerate((0, 3, 6)):
                            nc.tensor.matmul(
                                out=psj[:, n0: n0 + nw],
                                lhsT=band,
                                rhs=wb[:, n0 + s: n0 + s + nw],
                                start=(si == 0),
                                stop=(si == 2),
                            )

                # one wide PSUM->SBUF (fp32->fp16) evacuation on ScalarE
                nc.scalar.copy(out=ot[:, :], in_=psj[:, 0:W])
                # stores on GpSimd SWDGE: keeps both HWDGE rings for loads
                nc.gpsimd.dma_start(out=out_d[c, h0: h0 + BLK, :], in_=ot[:, :])
                tile_idx += 1


_NC = None


def _get_nc():
    global _NC
    if _NC is None:
        _NC = _build_nc()
    return _NC


def run(x: np.ndarray, trace: bool = False, trace_cores=None):
    """Run the kernel on all 8 cores. Returns (out, BassKernelResults)."""
    from concourse.bass_utils import run_bass_kernel_spmd

    nc = _get_nc()
    x = np.asarray(x)
    assert x.shape == (B, C, H, W), x.shape
    x16 = np.ascontiguousarray(x.astype(np.float16))
    in_maps = [{"x": x16[b]} for b in range(B)]
    if trace and trace_cores is None:
        trace_cores = [0, 7]
    res = run_bass_kernel_spmd(
        nc, in_maps, core_ids=list(range(B)), trace=trace, trace_cores=trace_cores
    )
    out = np.stack([res.results[b]["out"] for b in range(B)], axis=0)
    return out.astype(np.float32), res


def kernel(x: np.ndarray, r) -> np.ndarray:
    assert int(np.asarray(r)) == R, f"kernel hardcodes r={R}, got {r}"
    out, _ = run(x, trace=False)
    return out


# revision 12
# speedup vs baseline: 1.4777x; 1.0473x over previous
"""BoxFilter (9x9 box sum with edge clamping) on 8 Trainium2 NeuronCores.

Reference semantics (B, C, H, W fp32, r=4):
    out = diff_y(cumsum_W(diff_x(cumsum_H(x))))
i.e. a separable 9-wide box *sum* along H then W, with windows truncated at
the image borders.

Strategy (v4 — fp16 I/O, PE/DVE load-balanced W-box):
  - Shard data-parallel over batch: B=8 -> one (3, 1080, 1920) image per core.
  - HBM I/O in fp16 (host converts): halves DMA traffic vs fp32. The
    correctness budget (rel 2e-2 of output scale ~48) dwarfs fp16 rounding.
  - Per core, 27 tiles (3 channels x 9 blocks of 120 output rows); each
    tile loads 128 input rows (+-4 halo) x full W. The H-box is a constant
    0/1 banded matrix (top / interior / bottom variants) applied on PE.
  - W-box, two variants balanced across tiles:
      * comb path (most tiles): ones(9) = ones(3) conv {d(-3),d(0),d(+3)}.
        DVE computes box3 with two shifted tensor_tensor adds (fp16 2x_1p
        = 0.5 cyc/elem); PE applies the H-band 3x per 512-col group with
        rhs shifted by {0,3,6}, accumulating in PSUM (fp16 matmul is
        1 cyc/col, so 3 passes ~2.6us/tile).
      * scan path (a few tiles): the old DVE tensor_tensor_scan
        S[w] = S[w-1] + x[w+4] - x[w-5] (4.1us/tile, no fast mode) and a
        single PE pass. DVE cost ~2x comb, PE cost ~1/3: assigning ~3
        tiles to this path equalizes PE and DVE at ~64us each.
  - Zero-padded rows (9 left for scan warm-up, 4 right) give border
    truncation for free in both variants.
  - PSUM -> SBUF (fp32->fp16) as ONE wide ACTIVATE per tile on ScalarE
    ([120, 1920] over a 4-bank PSUM tile, bufs=2).
  - Loads: SP HWDGE ring mostly, every 3rd tile on the ACT ring; stores
    on GpSimd SWDGE; GpSimd is barred from PSUM so it only stores+memsets.
"""

import sys

if "/opt/trn_rl_repo" not in sys.path:
    sys.path.insert(0, "/opt/trn_rl_repo")

import numpy as np

B, C, H, W = 8, 3, 1080, 1920
R = 4
BLK = 120          # output rows per tile
NBLK = H // BLK    # 9
LP = 9             # left zero pad (scan warm-up needs 2r+1)
RP = 4             # right zero pad
XW = LP + W + RP   # padded row width (1933)
W3 = W + 6         # box3 row width: box3[w] for w in [-3, W+2]
SCN = W + R        # scan length; outputs [R:] are S[0..W-1]
N_TILES = C * NBLK
# tiles that take the scan path (DVE-heavy, PE-light) to balance engines
SCAN_TILES = frozenset({4, 13, 22})


def _band_matrices() -> np.ndarray:
    """[128, 3*BLK] fp16: the three 0/1 banded H-box matrices, side by side.

    out[m, n] = sum_k band[k, m] * in[k, n]; column m holds the taps for
    output row m of the block.
    """
    b0 = np.zeros((128, BLK), np.float16)   # first block: rows 0..127 loaded
    b1 = np.zeros((128, BLK), np.float16)   # interior: rows h0-4..h0+123
    b2 = np.zeros((128, BLK), np.float16)   # last block: rows H-128..H-1
    for m in range(BLK):
        b0[max(0, m - R): m + R + 1, m] = 1.0
        b1[m: m + 2 * R + 1, m] = 1.0
        b2[m + R: min(m + 3 * R, 127) + 1, m] = 1.0
    return np.concatenate([b0, b1, b2], axis=1)


def _build_nc():
    import concourse.tile as tile
    from concourse import bacc, mybir

    f16 = mybir.dt.float16
    nc = bacc.Bacc("TRN2", target_bir_lowering=False, debug=False)
    x_d = nc.dram_tensor("x", [C, H, W], f16, kind="ExternalInput").ap()
    out_d = nc.dram_tensor("out", [C, H, W], f16, kind="ExternalOutput").ap()
    bands_d = nc.inline_tensor(_band_matrices(), name="bands").ap()

    with tile.TileContext(nc) as tc:
        _tile_body(tc, out_d, x_d, bands_d, mybir)
    nc.compile()
    return nc


def _tile_body(tc, out_d, x_d, bands_d, mybir):
    nc = tc.nc
    add = mybir.AluOpType.add
    sub = mybir.AluOpType.subtract
    f16 = mybir.dt.float16
    f32 = mybir.dt.float32

    with (
        tc.tile_pool(name="bands", bufs=1) as bands_pool,
        tc.tile_pool(name="xp", bufs=10) as xpool,
        tc.tile_pool(name="t1", bufs=3) as tpool,
        tc.tile_pool(name="wb", bufs=4) as wpool,
        tc.tile_pool(name="ot", bufs=4) as opool,
        tc.tile_pool(name="ps", bufs=2, space="PSUM") as pspool,
    ):
        bands = bands_pool.tile([128, 3 * BLK], f16)
        first = True
        tile_idx = 0

        for c in range(C):
            for t in range(NBLK):
                h0 = t * BLK
                if t == 0:
                    r0, bi = 0, 0
                elif t == NBLK - 1:
                    r0, bi = H - 128, 2
                else:
                    r0, bi = h0 - R, 1

                xp = xpool.tile([128, XW], f16)
                if tile_idx < 10:
                    # pool buffers rotate round-robin; pads stay zero after
                    # the first pass since DMA only writes the middle.
                    # DVE, not GpSimd: GpSimd's preamble table-load ends
                    # late and would gate the first input loads
                    nc.vector.memset(xp[:, 0:LP], 0.0)
                    nc.vector.memset(xp[:, LP + W: XW], 0.0)
                if tile_idx < 2:
                    # ramp: split the first loads across both rings so the
                    # first tile's data lands in ~half the time
                    nc.sync.dma_start(
                        out=xp[0:64, LP: LP + W], in_=x_d[c, r0: r0 + 64, :]
                    )
                    nc.scalar.dma_start(
                        out=xp[64:128, LP: LP + W],
                        in_=x_d[c, r0 + 64: r0 + 128, :],
                    )
                else:
                    # most loads on the SP ring; every 3rd on the ACT ring so
                    # neither ring carries the whole 13.3 MB read stream
                    load_eng = nc.scalar if tile_idx % 3 == 2 else nc.sync
                    load_eng.dma_start(
                        out=xp[:, LP: LP + W], in_=x_d[c, r0: r0 + 128, :]
                    )
                if first:
                    nc.sync.dma_start(out=bands[:, :], in_=bands_d[:, :])
                    first = False

                band = bands[:, bi * BLK: (bi + 1) * BLK]
                ot = opool.tile([BLK, W], f16)
                psj = pspool.tile([BLK, 2048], f32)

                if tile_idx in SCAN_TILES:
                    # scan path: W-box in one DVE scan, one PE pass
                    wb = wpool.tile([128, W3], f16)
                    nc.vector.tensor_tensor_scan(
                        out=wb[:, 0:SCN],
                        data0=xp[:, LP: LP + SCN],
                        data1=xp[:, 0:SCN],
                        initial=0.0,
                        op0=add,
                        op1=sub,
                    )
                    for q in range(4):
                        n0 = q * 512
                        nw = min(512, W - n0)
                        nc.tensor.matmul(
                            out=psj[:, n0: n0 + nw],
                            lhsT=band,
                            rhs=wb[:, R + n0: R + n0 + nw],
                            start=True,
                            stop=True,
                        )
                else:
                    # comb path: box3 on DVE (2 adds), 3 shifted PE passes
                    # wb[:, j] = box3 at w=j-3  =  x[j-4] + x[j-3] + x[j-2]
                    #          = xp[:, j+5] + xp[:, j+6] + xp[:, j+7]
                    t1 = tpool.tile([128, W3], f16)
                    nc.vector.tensor_tensor(
                        out=t1[:, :], in0=xp[:, 5:5 + W3],
                        in1=xp[:, 6:6 + W3], op=add,
                    )
                    wb = wpool.tile([128, W3], f16)
                    nc.vector.tensor_tensor(
                        out=wb[:, :], in0=t1[:, :],
                        in1=xp[:, 7:7 + W3], op=add,
                    )
                    for q in range(4):
                        n0 = q * 512
                        nw = min(512, W - n0)
                        for si, s in enumerate((0, 3, 6)):
                            nc.tensor.matmul(
                                out=psj[:, n0: n0 + nw],
                                lhsT=band,
                                rhs=wb[:, n0 + s: n0 + s + nw],
                                start=(si == 0),
                                stop=(si == 2),
                            )

                # one wide PSUM->SBUF (fp32->fp16) evacuation on ScalarE
                nc.scalar.copy(out=ot[:, :], in_=psj[:, 0:W])
                # stores on GpSimd SWDGE: keeps both HWDGE rings for loads.
                # Last two tiles store via the (by then idle) HWDGE rings so
                # the end of the kernel isn't behind the SWDGE backlog+drain.
                if tile_idx == N_TILES - 1:
                    nc.scalar.dma_start(
                        out=out_d[c, h0: h0 + BLK, :], in_=ot[:, :]
                    )
                elif tile_idx == N_TILES - 2:
                    nc.sync.dma_start(
                        out=out_d[c, h0: h0 + BLK, :], in_=ot[:, :]
                    )
                else:
                    nc.gpsimd.dma_start(
                        out=out_d[c, h0: h0 + BLK, :], in_=ot[:, :]
                    )
                tile_idx += 1


_NC = None


def _get_nc():
    global _NC
    if _NC is None:
        _NC = _build_nc()
    return _NC


def run(x: np.ndarray, trace: bool = False, trace_cores=None):
    """Run the kernel on all 8 cores. Returns (out, BassKernelResults)."""
    from concourse.bass_utils import run_bass_kernel_spmd

    nc = _get_nc()
    x = np.asarray(x)
    assert x.shape == (B, C, H, W), x.shape
    x16 = np.ascontiguousarray(x.astype(np.float16))
    in_maps = [{"x": x16[b]} for b in range(B)]
    if trace and trace_cores is None:
        trace_cores = [0, 7]
    res = run_bass_kernel_spmd(
        nc, in_maps, core_ids=list(range(B)), trace=trace, trace_cores=trace_cores
    )
    out = np.stack([res.results[b]["out"] for b in range(B)], axis=0)
    return out.astype(np.float32), res


def kernel(x: np.ndarray, r) -> np.ndarray:
    assert int(np.asarray(r)) == R, f"kernel hardcodes r={R}, got {r}"
    out, _ = run(x, trace=False)
    return out
